# revision 19
# baseline (speedup 1.0000x reference)
"""Trainium2 Bass kernel for nn_BiDirectionalFusionModule.

Pure batch data-parallelism: 8 samples -> 8 NeuronCores, each core runs the
full module for one sample.

Big convs run as fp8(e4m3) DoubleRow matmuls (0.5 cyc/row, 256-deep
contraction = 4x bf16 column throughput):
 - conv1 (mask path): single-fp8 — quantization error is attenuated through
   sigmoid'(.)<=0.25 and the gamma=0.1-gated attention path.
 - fused conv2 (output): 3-term hi/lo split  w_hi(x_hi+x_lo) + w_lo x_hi,
   keeping near-bf16 accuracy at 3/4 of fp8 single-term cost.
 - SR 8x8/s8 convs and Q 1x1 convs: single-fp8.
Weights are pre-scaled by 64 on host so fp8 normals are used; the 1/64 is
folded into the PSUM-evict activation scales.

x arrives as byte-interleaved (hi,lo) fp8 [C,88,180]; conv1/SR/Q read the hi
bytes. The attention residual-apply streams the original bf16 x from DRAM per
22-row chunk, adds the LN'd attention output, and rewrites the interleaved
tile in place with enhanced (hi,lo) — which conv2 then consumes. Softmax
denominator cancels inside the following channel-LN (folded on host, as
before); LN stats via ones-column matmuls; per-pixel stats vectors reshaped
through DRAM to [121,16] tiles for wide DVE/ACT math.
"""
import numpy as np
import ml_dtypes
from contextlib import ExitStack

import concourse.bass as bass
from concourse import bacc
import concourse.tile as tile
import concourse.mybir as mybir
from concourse.bass_utils import run_bass_kernel_spmd

F32 = mybir.dt.float32
BF16 = mybir.dt.bfloat16
FP8 = mybir.dt.float8e4
AF = mybir.ActivationFunctionType
ALU = mybir.AluOpType
DR = mybir.MatmulPerfMode.DoubleRow
BF = ml_dtypes.bfloat16
E4 = ml_dtypes.float8_e4m3fn

B, C, H, W = 8, 256, 88, 88
RR = 8
HR = H // RR                # 11
M2 = HR * HR                # 121
N = H * W                   # 7744
PITCH = 90
WP = 2 * PITCH              # fp8 interleaved row pitch (bytes)
EPS = 1e-5
CQ = C // 8                 # 32
WS = 64.0                   # host-side fp8 weight scale

BLOCKS = [(i * 5, 5) for i in range(17)] + [(85, 3)]
CHUNK_ROWS = 22             # apply-phase chunking: 4 chunks of 22 rows

(CB_S1, CB_T1, CB_SRB0, CB_SRB1, CB_NG0, CB_NB0, CB_NG1, CB_NB1, CB_FS, CB_FT,
 CB_KB0, CB_QB0, CB_KB1, CB_QB1) = range(14)

_CACHE = {}


def _q8(a):
    return np.asarray(a, np.float32).astype(E4)


def _prep(inputs):
    ii = {k: np.asarray(v, dtype=np.float32) for k, v in inputs.items()}
    scale = float(CQ) ** -0.5

    def fold_bn(g, be, m, v):
        s = g / np.sqrt(v + EPS)
        return s, (0.0 - m) * s + be

    # conv1 (spatial-mask path): single fp8, weights x64
    w1T = ii['sm_w1'].transpose(2, 3, 1, 0).reshape(9, 2 * C, C)   # [o, cin, cout]
    w1_8 = np.zeros((128, 2, 9, 2, C), E4)
    for t in range(2):
        for s in range(2):
            w1_8[:, t, :, s, :] = _q8(WS * w1T[:, (2 * t + s) * 128:(2 * t + s + 1) * 128, :]
                                      ).transpose(1, 0, 2)
    s1, t1 = fold_bn(ii['sm_g1'], ii['sm_be1'], ii['sm_m1'], ii['sm_v1'])
    t1 = t1 + ii['sm_b1'] * s1
    w2T = ii['sm_w2'][:, :, 0, 0].T.astype(BF)
    b2 = float(ii['sm_b2'][0])

    # fused conv2: hi/lo fp8 split, weights x64
    fwT = ii['fus_w'][:, :2 * C].transpose(2, 3, 1, 0).reshape(9, 2 * C, C)
    fw_hi = _q8(WS * fwT)
    fw_lo = _q8(WS * fwT - fw_hi.astype(np.float32))
    fwA = np.zeros((128, 4, 9, 2, C), E4)
    for ci in range(4):
        for s in range(2):
            fwA[:, ci, :, s, :] = fw_hi[:, ci * 128:(ci + 1) * 128, :].transpose(1, 0, 2)
    fwB = np.zeros((128, 2, 9, 2, C), E4)
    for p in range(2):
        for s in range(2):
            fwB[:, p, :, s, :] = fw_lo[:, (2 * p + s) * 128:(2 * p + s + 1) * 128, :
                                       ].transpose(1, 0, 2)
    # x WS: shares a PSUM group with the 64x-scaled fp8 terms (evict divides by WS)
    fwm = (WS * ii['fus_w'][:, 2 * C, :, :]).transpose(1, 2, 0).reshape(9, C).astype(BF)
    fs, ft = fold_bn(ii['fus_g'], ii['fus_be'], ii['fus_m'], ii['fus_v'])
    ft = ft + ii['fus_b'] * fs

    dirs = {}
    for di, pfx in enumerate(('d2r', 'r2d')):
        g = ii[pfx + '_ln_g']; bl = ii[pfx + '_ln_b']
        kw = ii[pfx + '_k_w'][:, :, 0, 0]; kb = ii[pfx + '_k_b']
        vw = ii[pfx + '_v_w'][:, :, 0, 0]; vb = ii[pfx + '_v_b']
        qw = ii[pfx + '_q_w'][:, :, 0, 0]; qb = ii[pfx + '_q_b']
        gamma = float(np.clip(ii[pfx + '_gamma'], 0.0, 1.0)[0])
        srwT = ii[pfx + '_sr_w'].transpose(2, 3, 1, 0).reshape(64, C, C)
        srw8 = np.zeros((128, 4, 2, 16, C), E4)
        for grp in range(4):
            for s in range(2):
                srw8[:, grp, s, :, :] = _q8(
                    WS * srwT[grp * 16:(grp + 1) * 16, s * 128:(s + 1) * 128, :]
                ).transpose(1, 0, 2)
        dirs[di] = dict(
            srw8=srw8, srb=ii[pfx + '_sr_b'],
            kwT=(scale * kw * g[None, :]).T.astype(BF),
            kb=scale * (kb + kw @ bl),
            qwT=qw.T, qb=qb,
            vwN=(vw * g[None, :]).T.astype(BF),
            vb=(vb + vw @ bl).astype(BF),
            ng=gamma * ii[pfx + '_norm_g'],
            nb=gamma * ii[pfx + '_norm_b'],
        )

    cb = np.zeros((C, 14), np.float32)
    cb[:, CB_S1] = s1 / WS; cb[:, CB_T1] = t1
    cb[:, CB_SRB0] = dirs[0]['srb']; cb[:, CB_SRB1] = dirs[1]['srb']
    cb[:, CB_NG0] = dirs[0]['ng']; cb[:, CB_NB0] = dirs[0]['nb']
    cb[:, CB_NG1] = dirs[1]['ng']; cb[:, CB_NB1] = dirs[1]['nb']
    cb[:, CB_FS] = fs / WS; cb[:, CB_FT] = ft
    cb[:CQ, CB_KB0] = dirs[0]['kb']; cb[:CQ, CB_QB0] = dirs[0]['qb']
    cb[:CQ, CB_KB1] = dirs[1]['kb']; cb[:CQ, CB_QB1] = dirs[1]['qb']
    cbp = np.zeros((128, 28), np.float32)
    cbp[:, 0:14] = cb[0:128]; cbp[:, 14:28] = cb[128:256]

    # bf16 K-projection weights (contract over kvn), per cb-half
    kqk = np.zeros((C, 2 * CQ), BF)
    kqk[:, 0:CQ] = dirs[0]['kwT']; kqk[:, CQ:] = dirs[1]['kwT']
    # fp8 Q-projection weights (contract over x/msk hi), x64
    kq8 = np.zeros((128, 2, 2 * CQ), E4)
    for s in range(2):
        for di in range(2):
            kq8[:, s, di * CQ:(di + 1) * CQ] = _q8(
                WS * dirs[di]['qwT'][s * 128:(s + 1) * 128, :])
    vw2 = np.concatenate([dirs[0]['vwN'], dirs[1]['vwN']], axis=1)
    vbr = np.concatenate([dirs[0]['vb'], dirs[1]['vb']])[None, :]

    shared = dict(w1=w1_8.reshape(128, -1), fwa=fwA.reshape(128, -1),
                  fwb=fwB.reshape(128, -1), w2=w2T, fwm=fwm, cb=cbp,
                  kqk=np.ascontiguousarray(kqk), kq8=kq8.reshape(128, -1),
                  vw2=np.ascontiguousarray(vw2), vbr=np.ascontiguousarray(vbr),
                  srw0=dirs[0]['srw8'].reshape(128, -1),
                  srw1=dirs[1]['srw8'].reshape(128, -1))

    rgb = ii['f_rgb']; dep = ii['f_depth']
    in_maps = []
    for i in range(B):
        full = np.concatenate([rgb[i], dep[i]], axis=0)        # [512, 88, 88]
        x8 = np.zeros((2 * C, H, WP), E4)
        x8[:, :, 2:178:2] = full.astype(E4)
        xb = np.zeros((2 * C, H, PITCH), BF)
        xb[:, :, 1:89] = full.astype(BF)
        m = dict(shared)
        m['x8'] = np.ascontiguousarray(x8.reshape(2 * C, H * WP))
        m['xb'] = np.ascontiguousarray(xb.reshape(2 * C, H * PITCH))
        in_maps.append(m)
    return in_maps, b2


def _build(nc, b2, dbg=False, maxphase=4):
    x8_d = nc.dram_tensor("x8", [2 * C, H * WP], FP8, kind="ExternalInput")
    xb_d = nc.dram_tensor("xb", [2 * C, H * PITCH], BF16, kind="ExternalInput")
    w1_d = nc.dram_tensor("w1", [128, 2 * 9 * 2 * C], FP8, kind="ExternalInput")
    fwa_d = nc.dram_tensor("fwa", [128, 4 * 9 * 2 * C], FP8, kind="ExternalInput")
    fwb_d = nc.dram_tensor("fwb", [128, 2 * 9 * 2 * C], FP8, kind="ExternalInput")
    w2_d = nc.dram_tensor("w2", [C, 1], BF16, kind="ExternalInput")
    fwm_d = nc.dram_tensor("fwm", [9, C], BF16, kind="ExternalInput")
    cb_d = nc.dram_tensor("cb", [128, 28], F32, kind="ExternalInput")
    kqk_d = nc.dram_tensor("kqk", [C, 2 * CQ], BF16, kind="ExternalInput")
    kq8_d = nc.dram_tensor("kq8", [128, 2 * 2 * CQ], FP8, kind="ExternalInput")
    vw2_d = nc.dram_tensor("vw2", [C, 2 * C], BF16, kind="ExternalInput")
    vbr_d = nc.dram_tensor("vbr", [1, 2 * C], BF16, kind="ExternalInput")
    srw_d = [nc.dram_tensor("srw0", [128, 4 * 2 * 16 * C], FP8, kind="ExternalInput"),
             nc.dram_tensor("srw1", [128, 4 * 2 * 16 * C], FP8, kind="ExternalInput")]
    out_d = nc.dram_tensor("out", [C, N], F32, kind="ExternalOutput")
    dbg_d = {}
    if dbg:
        for nm, shp in [("mask", [1, H * PITCH]),
                        ("kvr0", [C, M2]), ("kvr1", [C, M2]),
                        ("kvn0", [C, M2]), ("kvn1", [C, M2]),
                        ("k0", [32, M2]), ("k1", [32, M2]),
                        ("v0", [M2, C]), ("v1", [M2, C]),
                        ("num0", [C, N]), ("num1", [C, N]),
                        ("rm0", [2, N]), ("rm1", [2, N])]:
            dbg_d[nm] = nc.dram_tensor("dbg_" + nm, shp, BF16, kind="ExternalOutput")
        dbg_d["ehi"] = nc.dram_tensor("dbg_ehi", [2 * C, N], BF16, kind="ExternalOutput")
        dbg_d["elo"] = nc.dram_tensor("dbg_elo", [2 * C, N], BF16, kind="ExternalOutput")

    with tile.TileContext(nc) as tc:
        es = ExitStack()
        with es, tc.tile_pool(name="dram", bufs=1, space="DRAM") as dpool:
            gp = es.enter_context(tc.tile_pool(name="gp", bufs=1))

            cb_sb = gp.tile([128, 28], F32, name="cb_sb")

            def cbc(col, half):
                return cb_sb[:, half * 14 + col:half * 14 + col + 1]

            kqk_sb = gp.tile([128, 2, 2 * CQ], BF16, name="kqk_sb")
            kq8_sb = gp.tile([128, 2, 2 * CQ], FP8, name="kq8_sb")
            vw2_sb = gp.tile([128, 2, 2 * C], BF16, name="vw2_sb")
            vbr_sb = gp.tile([1, 2 * C], BF16, name="vbr_sb")
            w2_sb = gp.tile([128, 2, 1], BF16, name="w2_sb")
            ones_bf = gp.tile([128, 1], BF16, name="ones_bf")
            nc.vector.memset(ones_bf, 1.0)
            ones1_bf = gp.tile([1, M2], BF16, name="ones1_bf")
            nc.vector.memset(ones1_bf, 1.0)
            zrow = gp.tile([1, PITCH], BF16, name="zrow")
            nc.vector.memset(zrow, 0.0)
            eps_sb = gp.tile([128, 1], F32, name="eps_sb")
            nc.vector.memset(eps_sb, EPS)
            b2_sb = gp.tile([128, 1], F32, name="b2_sb")
            nc.vector.memset(b2_sb, b2)

            mask_dram = dpool.tile([1, PITCH * PITCH], BF16, name="mask_dram")
            pool_x = es.enter_context(tc.tile_pool(name="px", bufs=1))
            # interleaved (hi,lo) fp8 input; rewritten in place with the
            # enhanced features by the phase-3 residual apply
            xi8 = pool_x.tile([128, 4, H, WP], FP8, name="xi8")
            x8v = x8_d.rearrange("(t p) (h q) -> t p h q", p=128, q=WP)

            def hi_v(ci, rlo, rhi, dx):
                # [128, nr, 88] hi-byte view at conv offset dx
                return xi8[:, ci, rlo:rhi, 2 * dx:2 * dx + 176:2]

            def hi_pair(pr, rlo, rhi, dx):
                # [128, 2, nr, 88] hi bytes for ci pair
                return xi8[:, 2 * pr:2 * pr + 2, rlo:rhi, 2 * dx:2 * dx + 176:2]

            with tc.tile_pool(name="pmsk", bufs=1) as pmsk:
             with tc.tile_pool(name="srp", bufs=5) as srp:
              # ============== Phase 1: conv1 + spatial mask ==============
              with tc.tile_pool(name="pms", bufs=1) as pms:
                mask_sb = pms.tile([1, H, PITCH], BF16, name="mask_sb")
                nc.gpsimd.memset(mask_sb, 0.0)
                mask3 = mask_sb  # [1, 88, 90]
                with tc.tile_pool(name="pw1", bufs=1) as pw1, \
                     tc.tile_pool(name="ps1", bufs=3, space="PSUM") as ps1, \
                     tc.tile_pool(name="ps1m", bufs=2, space="PSUM") as ps1m, \
                     tc.tile_pool(name="ev1", bufs=2) as ev:
                    if maxphase < 1:
                        return
                    # weights first: the first conv matmul gates on these
                    w1_sb = pw1.tile([128, 2, 9, 2, C], FP8, name="w1_sb")
                    nc.sync.dma_start(
                        out=w1_sb,
                        in_=w1_d.rearrange("p (t o s c) -> p t o s c", t=2, o=9, s=2))
                    for ci in range(4):
                        nc.sync.dma_start(out=xi8[:, ci, 0:44, :],
                                          in_=x8v[ci][:, 0:44, :])
                    nc.sync.dma_start(out=cb_sb, in_=cb_d[:, :])
                    nc.sync.dma_start(out=kqk_sb,
                                      in_=kqk_d.rearrange("(s p) q -> p s q", p=128))
                    nc.sync.dma_start(out=kq8_sb,
                                      in_=kq8_d.rearrange("p (s q) -> p s q", s=2))
                    for t in range(2):
                        nc.sync.dma_start(out=vw2_sb[:, t, :],
                                          in_=vw2_d.rearrange("(t p) q -> t p q", p=128)[t])
                    nc.sync.dma_start(out=vbr_sb, in_=vbr_d[:, :])
                    for t in range(2):
                        nc.sync.dma_start(out=w2_sb[:, t, :],
                                          in_=w2_d.rearrange("(t p) q -> t p q", p=128)[t])
                    for ci in range(4):
                        nc.sync.dma_start(out=xi8[:, ci, 44:88, :],
                                          in_=x8v[ci][:, 44:88, :])

                    for y0, nr in BLOCKS:
                        nn = nr * W
                        h1b = []
                        for cb_i in range(2):
                            ps = ps1.tile([128, nr, W], F32, name="c1ps", tag="c1ps")
                            psf = ps.rearrange("p r w -> p (r w)")
                            plan = _c3plan(y0, nr, 2)
                            for i, (o, t, s, ylo, yhi) in enumerate(plan):
                                out = psf if (ylo == y0 and yhi == y0 + nr) else \
                                    psf[:, (ylo - y0) * W:(yhi - y0) * W]
                                nc.tensor.matmul(
                                    out,
                                    w1_sb[:, t, o, :, cb_i * 128:(cb_i + 1) * 128],
                                    hi_pair(t, ylo + s, yhi + s, o % 3),
                                    start=(i == 0), stop=(i == len(plan) - 1),
                                    perf_mode=DR)
                            h1t = ev.tile([128, nn], BF16, name="h1t", tag=f"h1t{cb_i}")
                            nc.scalar.activation(h1t, psf, AF.Relu,
                                                 bias=cbc(CB_T1, cb_i),
                                                 scale=cbc(CB_S1, cb_i))
                            h1b.append(h1t)
                        mps = ps1m.tile([1, nn], F32, name="mps", tag="mps")
                        for cb_i in range(2):
                            nc.tensor.matmul(mps, w2_sb[:, cb_i, :], h1b[cb_i],
                                             start=(cb_i == 0), stop=(cb_i == 1))
                        nc.scalar.activation(mask3[:, y0:y0 + nr, 1:89], mps,
                                             AF.Sigmoid, bias=b2_sb[0:1, :], scale=1.0)
                # prefetch r2d srw chunks ahead of the mask-gated work
                preload = {}
                for grp in range(4):
                    wp = srp.tile([128, 2, 16, C], FP8, name="wch", tag="wch")
                    nc.sync.dma_start(out=wp, in_=srw_d[1].rearrange(
                        "p (g s o c) -> p g s o c", g=4, s=2, o=16)[:, grp])
                    preload[grp] = wp
                # mask -> zero-padded 90x90 in DRAM
                nc.sync.dma_start(out=mask_dram[:, 0:PITCH], in_=zrow)
                nc.sync.dma_start(out=mask_dram[:, 89 * PITCH:], in_=zrow)
                nc.sync.dma_start(out=mask_dram[:, PITCH:89 * PITCH],
                                  in_=mask_sb.rearrange("o h q -> o (h q)"))
                if dbg:
                    nc.sync.dma_start(out=dbg_d["mask"][:, :],
                                      in_=mask_sb.rearrange("o h q -> o (h q)"))
              if maxphase < 2:
                  return
              with tc.tile_pool(name="pmb", bufs=1) as pmb:
                  mask_b = pmb.tile([128, H, W], BF16, name="mask_b")
                  m90 = mask_dram.rearrange("o (h q) -> o h q", q=PITCH)
                  nc.sync.dma_start(out=mask_b,
                                    in_=m90[:, 1:89, 1:89].to_broadcast([128, H, W]))
                  msk8 = pmsk.tile([128, 2, H, W], FP8, name="msk8")
                  for t in range(2):
                      nc.vector.tensor_tensor(out=msk8[:, t],
                                              in1=xi8[:, 2 + t, :, 2:178:2],
                                              in0=mask_b, op=ALU.mult)

              # ====== Phase 2: sr-conv + channel-LN + K / V^T (r2d then d2r) ======
              kvs = {}
              with tc.tile_pool(name="ps2", bufs=1, space="PSUM") as ps2, \
                   tc.tile_pool(name="ps2s", bufs=1, space="PSUM") as ps2s, \
                   tc.tile_pool(name="ev2", bufs=2) as ev:
                  for di in (1, 0):
                      if di == 0:
                          kvf_view = lambda dy, dx: msk8[:, :, dy::RR, dx::RR]
                      else:
                          kvf_view = lambda dy, dx: \
                              xi8[:, 0:2, dy::RR, 2 + 2 * dx:2 + 2 * dx + 162:16]
                      srps = [ps2.tile([128, M2], F32, name="srps", tag=f"srps{i}")
                              for i in range(2)]
                      for grp in range(4):
                          if di == 1:
                              wch = preload[grp]
                          else:
                              wch = srp.tile([128, 2, 16, C], FP8, name="wch",
                                             tag="wch")
                              nc.sync.dma_start(out=wch, in_=srw_d[di].rearrange(
                                  "p (g s o c) -> p g s o c", g=4, s=2, o=16)[:, grp])
                          for o in range(16):
                              off = grp * 16 + o
                              rhs = kvf_view(off // 8, off % 8)
                              for cb_i in range(2):
                                  nc.tensor.matmul(
                                      srps[cb_i],
                                      wch[:, :, o, cb_i * 128:(cb_i + 1) * 128],
                                      rhs,
                                      start=(off == 0), stop=(off == 63),
                                      perf_mode=DR)
                      kvr = []
                      for cb_i in range(2):
                          kt = ev.tile([128, M2], BF16, name="kvr", tag=f"kvr{cb_i}")
                          nc.scalar.activation(kt, srps[cb_i], AF.Identity,
                                               bias=cbc(CB_SRB0 + di, cb_i),
                                               scale=1.0 / WS)
                          kvr.append(kt)
                          if dbg:
                              nc.sync.dma_start(
                                  out=dbg_d[f"kvr{di}"][cb_i * 128:(cb_i + 1) * 128, :],
                                  in_=kt)
                      mu_ps = ps2s.tile([1, M2], F32, name="mups", tag="mups")
                      sq_ps = ps2s.tile([1, M2], F32, name="sqps", tag="sqps")
                      for cb_i in range(2):
                          sq = ev.tile([128, M2], BF16, name="sqkv", tag="sqkv")
                          nc.vector.tensor_tensor(out=sq, in0=kvr[cb_i], in1=kvr[cb_i],
                                                  op=ALU.mult)
                          nc.tensor.matmul(mu_ps, ones_bf, kvr[cb_i],
                                           start=(cb_i == 0), stop=(cb_i == 1))
                          nc.tensor.matmul(sq_ps, ones_bf, sq,
                                           start=(cb_i == 0), stop=(cb_i == 1))
                      mu = ev.tile([1, M2], F32, name="mukv", tag="mukv")
                      nc.vector.tensor_scalar(mu, mu_ps, 1.0 / C, None, ALU.mult)
                      ms = ev.tile([1, M2], F32, name="mskv", tag="mskv")
                      nc.vector.tensor_scalar(ms, sq_ps, 1.0 / C, None, ALU.mult)
                      mu2 = ev.tile([1, M2], F32, name="mu2kv", tag="mu2kv")
                      nc.vector.tensor_tensor(out=mu2, in0=mu, in1=mu, op=ALU.mult)
                      nc.vector.tensor_tensor(out=ms, in0=ms, in1=mu2, op=ALU.subtract)
                      sd = ev.tile([1, M2], F32, name="sdkv", tag="sdkv")
                      nc.scalar.activation(sd, ms, AF.Sqrt, bias=eps_sb[0:1, :],
                                           scale=1.0)
                      rstd = ev.tile([1, M2], F32, name="rstdkv", tag="rstdkv")
                      nc.vector.reciprocal(rstd, sd)
                      nrm_bf = ev.tile([1, 2, M2], BF16, name="nrmbf", tag="nrmbf")
                      nc.vector.tensor_copy(nrm_bf[:, 0, :], rstd)
                      murm = ev.tile([1, M2], F32, name="murm", tag="murm")
                      nc.vector.tensor_tensor(out=murm, in0=mu, in1=rstd, op=ALU.mult)
                      nc.vector.tensor_copy(nrm_bf[:, 1, :], murm)
                      nrm_dram = dpool.tile([2, M2], BF16, name="nrm_dram",
                                            tag="nrm_dram", bufs=2)
                      nc.sync.dma_start(out=nrm_dram[0:1, :], in_=nrm_bf[:, 0, :])
                      nc.sync.dma_start(out=nrm_dram[1:2, :], in_=nrm_bf[:, 1, :])
                      rstd_b = ev.tile([128, M2], BF16, name="rstdb", tag="rstdb")
                      nc.sync.dma_start(out=rstd_b,
                                        in_=nrm_dram[0:1, :].to_broadcast([128, M2]))
                      mur_b = ev.tile([128, M2], BF16, name="murb", tag="murb")
                      nc.sync.dma_start(out=mur_b,
                                        in_=nrm_dram[1:2, :].to_broadcast([128, M2]))
                      kvn = []
                      for cb_i in range(2):
                          kn = gp.tile([128, M2], BF16, name=f"kvn{di}{cb_i}")
                          nc.vector.tensor_tensor(out=kn, in0=kvr[cb_i], in1=rstd_b,
                                                  op=ALU.mult)
                          nc.vector.tensor_tensor(out=kn, in0=kn, in1=mur_b,
                                                  op=ALU.subtract)
                          kvn.append(kn)
                          if dbg:
                              nc.sync.dma_start(
                                  out=dbg_d[f"kvn{di}"][cb_i * 128:(cb_i + 1) * 128, :],
                                  in_=kn)
                      kps = ps2s.tile([32, M2], F32, name="kps", tag="kps")
                      for cb_i in range(2):
                          nc.tensor.matmul(kps,
                                           kqk_sb[:, cb_i, di * CQ:(di + 1) * CQ],
                                           kvn[cb_i], start=(cb_i == 0),
                                           stop=(cb_i == 1))
                      k_bf = gp.tile([32, M2], BF16, name=f"k_bf{di}")
                      nc.scalar.activation(
                          k_bf, kps, AF.Identity,
                          bias=cb_sb[0:32, CB_KB0 + 2 * di:CB_KB0 + 2 * di + 1],
                          scale=1.0)
                      vps = ps2.tile([M2, C], F32, name="vps", tag="vps")
                      for cb_i in range(2):
                          nc.tensor.matmul(vps, kvn[cb_i],
                                           vw2_sb[:, cb_i, di * C:(di + 1) * C],
                                           start=(cb_i == 0), stop=False)
                      nc.tensor.matmul(vps, ones1_bf, vbr_sb[:, di * C:(di + 1) * C],
                                       start=False, stop=True)
                      v_bf = gp.tile([M2, C], BF16, name=f"v_bf{di}")
                      vcol = ev.tile([M2, 1], F32, name="vcol", tag="vcol")
                      nc.scalar.activation(v_bf, vps, AF.Identity, accum_out=vcol)
                      vc_bf = gp.tile([M2, 1], BF16, name=f"vc_bf{di}")
                      nc.vector.tensor_copy(vc_bf, vcol)
                      if dbg:
                          nc.sync.dma_start(out=dbg_d[f"k{di}"][:, :], in_=k_bf)
                          nc.sync.dma_start(out=dbg_d[f"v{di}"][:, :], in_=v_bf)
                      kvs[di] = (k_bf, v_bf, vc_bf)

             # ====== Phase 3: attention + LN + residual (r2d then d2r) ======
             if maxphase < 3:
                 return
             pfw_es = ExitStack()
             pfw_box = {}

             def load_fw():
                 pfw = pfw_es.enter_context(tc.tile_pool(name="pfw", bufs=1, side="right"))
                 fwa_sb = pfw.tile([128, 4, 9, 2, C], FP8, name="fwa_sb")
                 nc.sync.dma_start(
                     out=fwa_sb,
                     in_=fwa_d.rearrange("p (ci o s c) -> p ci o s c", ci=4, o=9, s=2))
                 fwb_sb = pfw.tile([128, 2, 9, 2, C], FP8, name="fwb_sb")
                 nc.sync.dma_start(
                     out=fwb_sb,
                     in_=fwb_d.rearrange("p (t o s c) -> p t o s c", t=2, o=9, s=2))
                 fwm_sb = pfw.tile([9, C], BF16, name="fwm_sb")
                 nc.sync.dma_start(out=fwm_sb, in_=fwm_d[:, :])
                 im2 = pfw.tile([9, PITCH * PITCH], BF16, name="im2")
                 nc.gpsimd.memset(im2, 0.0)
                 for dy in range(3):
                     for dx in range(3):
                         j = dy * 3 + dx
                         joff = dy * PITCH + dx
                         nc.sync.dma_start(
                             out=im2[j:j + 1, 0:PITCH * PITCH - joff],
                             in_=mask_dram[:, joff:])
                 pfw_box.update(fwa_sb=fwa_sb, fwb_sb=fwb_sb, fwm_sb=fwm_sb, im2=im2)
             xbv = xb_d.rearrange("(t p) (h q) -> t p h q", p=128, q=PITCH)
             with tc.tile_pool(name="ps3", bufs=1, space="PSUM") as ps3, \
                  tc.tile_pool(name="ps3n", bufs=1, space="PSUM") as ps3n, \
                  tc.tile_pool(name="ev3", bufs=2) as ev, \
                  tc.tile_pool(name="nump", bufs=1) as num_p, \
                  tc.tile_pool(name="xcp", bufs=2) as xc_p, \
                  tc.tile_pool(name="rbp", bufs=1) as rb_p:
                num_d, dram_d = {}, {}

                def emit_blocks(di, npool):
                    stats_dram = dpool.tile([2, N], F32, name=f"stats_dram{di}",
                                            tag="stats_dram", bufs=2)
                    rmur_dram = dpool.tile([2, N], BF16, name=f"rmur_dram{di}",
                                           tag="rmur_dram", bufs=2)
                    num_sb = [npool.tile([128, N], BF16, name=f"num{di}{cb_i}",
                                         tag=f"num{di}{cb_i}") for cb_i in range(2)]
                    num_d[di] = num_sb
                    dram_d[di] = (stats_dram, rmur_dram)
                    k_bf, v_bf, vc_bf = kvs[di]
                    for bi, (y0, nr) in enumerate(BLOCKS):
                        nn = nr * W
                        qps = ps3n.tile([32, nn], F32, name="qps", tag="qps")
                        rhs = (msk8[:, :, y0:y0 + nr, :] if di == 1
                               else hi_pair(0, y0, y0 + nr, 1))
                        nc.tensor.matmul(qps,
                                         kq8_sb[:, :, di * CQ:(di + 1) * CQ],
                                         rhs, start=True, stop=True, perf_mode=DR)
                        q_bf = ev.tile([32, nn], BF16, name="q_bf", tag="q_bf")
                        nc.scalar.activation(
                            q_bf, qps, AF.Identity,
                            bias=cb_sb[0:32, CB_QB0 + 2 * di:CB_QB0 + 2 * di + 1],
                            scale=1.0 / WS)
                        sps = ps3.tile([M2, nn], F32, name="sps", tag="sps")
                        nc.tensor.matmul(sps, k_bf, q_bf, start=True, stop=True)
                        e_bf = ev.tile([M2, nn], BF16, name="e_bf", tag="e_bf")
                        nc.scalar.activation(e_bf, sps, AF.Exp)
                        mu_ps = ps3n.tile([1, nn], F32, name="amups", tag="astps",
                                          bufs=2)
                        nc.tensor.matmul(mu_ps, vc_bf, e_bf, start=True, stop=True)
                        sq_ps = ps3n.tile([1, nn], F32, name="asqps", tag="astps",
                                          bufs=2)
                        for cb_i in range(2):
                            nps = ps3.tile([128, nn], F32, name="nps",
                                           tag=f"nps{cb_i}", bufs=2)
                            nc.tensor.matmul(nps,
                                             v_bf[:, cb_i * 128:(cb_i + 1) * 128],
                                             e_bf, start=True, stop=True)
                            nc.vector.tensor_copy(num_sb[cb_i][:, y0 * W:y0 * W + nn],
                                                  nps)
                            nsq = ev.tile([128, nn], BF16, name="nsq", tag="nsq")
                            if cb_i == 0:
                                nc.scalar.activation(nsq, nps, AF.Square)
                            else:
                                segq = num_sb[cb_i][:, y0 * W:y0 * W + nn]
                                nc.vector.tensor_tensor(out=nsq, in0=segq, in1=segq,
                                                        op=ALU.mult)
                            nc.tensor.matmul(sq_ps, ones_bf, nsq,
                                             start=(cb_i == 0), stop=(cb_i == 1))
                        mrow = ev.tile([1, nn], F32, name="mrow", tag="mrow")
                        nc.vector.tensor_scalar(mrow, mu_ps, 1.0 / C, None, ALU.mult)
                        nc.sync.dma_start(out=stats_dram[0:1, y0 * W:y0 * W + nn],
                                          in_=mrow)
                        srow = ev.tile([1, nn], F32, name="srow", tag="srow")
                        nc.scalar.activation(srow, sq_ps, AF.Identity, scale=1.0 / C)
                        nc.sync.dma_start(out=stats_dram[1:2, y0 * W:y0 * W + nn],
                                          in_=srow)
                    if dbg:
                        for cb_i in range(2):
                            nc.sync.dma_start(
                                out=dbg_d[f"num{di}"][cb_i * 128:(cb_i + 1) * 128, :],
                                in_=num_sb[cb_i])

                def emit_apply(di, ch):
                    stats_dram, rmur_dram = dram_d[di]
                    num_sb = num_d[di]
                    c0 = ch * CHUNK_ROWS * W
                    cn = CHUNK_ROWS * W
                    rows = slice(ch * CHUNK_ROWS, (ch + 1) * CHUNK_ROWS)
                    mu_t = ev.tile([121, 16], F32, name="mu_t", tag="mu_t")
                    nc.sync.dma_start(
                        out=mu_t,
                        in_=stats_dram[0, c0:c0 + cn].rearrange("(p j) -> p j", j=16))
                    ms_t = ev.tile([121, 16], F32, name="ms_t", tag="ms_t")
                    nc.sync.dma_start(
                        out=ms_t,
                        in_=stats_dram[1, c0:c0 + cn].rearrange("(p j) -> p j", j=16))
                    mu2_t = ev.tile([121, 16], F32, name="mu2_t", tag="mu2_t")
                    nc.vector.tensor_tensor(out=mu2_t, in0=mu_t, in1=mu_t,
                                            op=ALU.mult)
                    nc.vector.tensor_tensor(out=ms_t, in0=ms_t, in1=mu2_t,
                                            op=ALU.subtract)
                    sd_t = ev.tile([121, 16], F32, name="sd_t", tag="sd_t")
                    nc.scalar.activation(sd_t, ms_t, AF.Sqrt,
                                         bias=eps_sb[0:121, :], scale=1.0)
                    r_t = ev.tile([121, 16], F32, name="r_t", tag="r_t")
                    nc.vector.reciprocal(r_t, sd_t)
                    rm_bf = ev.tile([121, 2, 16], BF16, name="rm_bf", tag="rm_bf")
                    nc.vector.tensor_copy(rm_bf[:, 0, :], r_t)
                    nc.vector.tensor_tensor(out=mu_t, in0=mu_t, in1=r_t,
                                            op=ALU.mult)
                    nc.vector.tensor_copy(rm_bf[:, 1, :], mu_t)
                    nc.sync.dma_start(
                        out=rmur_dram[:, c0:c0 + cn]
                        .rearrange("t (p j) -> p t j", j=16), in_=rm_bf)
                    r_b = rb_p.tile([128, cn], BF16, name="r_b", tag="r_b")
                    nc.sync.dma_start(
                        out=r_b,
                        in_=rmur_dram[0:1, c0:c0 + cn].to_broadcast([128, cn]))
                    mur_b = rb_p.tile([128, cn], BF16, name="mur_b", tag="mur_b")
                    nc.sync.dma_start(
                        out=mur_b,
                        in_=rmur_dram[1:2, c0:c0 + cn].to_broadcast([128, cn]))
                    for cb_i in range(2):
                        ci = 2 * di + cb_i
                        xc = xc_p.tile([128, CHUNK_ROWS, W], BF16, name="xc",
                                       tag="xc")
                        nc.sync.dma_start(out=xc, in_=xbv[ci][:, rows, 1:89])
                        seg = num_sb[cb_i][:, c0:c0 + cn]
                        nc.vector.tensor_tensor(out=seg, in0=seg, in1=r_b,
                                                op=ALU.mult)
                        nc.vector.tensor_tensor(out=seg, in0=seg, in1=mur_b,
                                                op=ALU.subtract)
                        nc.scalar.activation(seg, seg, AF.Identity,
                                             bias=cbc(CB_NB0 + 2 * di, cb_i),
                                             scale=cbc(CB_NG0 + 2 * di, cb_i))
                        t_sb = ev.tile([128, CHUNK_ROWS, W], BF16, name="t_sb",
                                       tag="t_sb")
                        nc.vector.tensor_tensor(
                            out=t_sb,
                            in0=num_sb[cb_i]
                            .rearrange("p (h w) -> p h w", w=W)[:, rows, :],
                            in1=xc, op=ALU.add)
                        hi_w = xi8[:, ci, rows, 2:178:2]
                        lo_w = xi8[:, ci, rows, 3:179:2]
                        nc.scalar.activation(hi_w, t_sb, AF.Identity)
                        nc.gpsimd.tensor_tensor(out=lo_w, in0=t_sb, in1=hi_w,
                                                op=ALU.subtract)

                with tc.tile_pool(name="np1", bufs=1) as np1:
                    emit_blocks(1, np1)
                    emit_blocks(0, num_p)
                    for ch in range(4):
                        emit_apply(1, ch)
                    if dbg:
                        nc.sync.dma_start(out=dbg_d["rm1"][:, :], in_=dram_d[1][1])
                load_fw()
                for ch in range(4):
                    emit_apply(0, ch)
                if dbg:
                    nc.sync.dma_start(out=dbg_d["rm0"][:, :], in_=dram_d[0][1])

             # ================= Phase 4: conv2 =================
             if maxphase < 4:
                 return
             if dbg:
                 with tc.tile_pool(name="dbgp", bufs=2) as dbgp:
                     for ci in range(4):
                         for nm, off in (("ehi", 2), ("elo", 3)):
                             st = dbgp.tile([128, H, W], BF16, name="dbg_st",
                                            tag="dbg_st")
                             nc.vector.tensor_copy(
                                 st, xi8[:, ci, :, off:off + 176:2])
                             nc.sync.dma_start(
                                 out=dbg_d[nm][ci * 128:(ci + 1) * 128, :],
                                 in_=st.rearrange("p h w -> p (h w)"))
             fwa_sb = pfw_box["fwa_sb"]; fwb_sb = pfw_box["fwb_sb"]
             fwm_sb = pfw_box["fwm_sb"]; im2 = pfw_box["im2"]
             with tc.tile_pool(name="ps4", bufs=4, space="PSUM") as ps4, \
                  tc.tile_pool(name="ev4", bufs=2) as ev:
                 im2v = im2.rearrange("o (h q) -> o h q", q=PITCH)

                 for y0, nr in BLOCKS:
                     nn = nr * W
                     for cb_i in range(2):
                         ps = ps4.tile([128, nr, W], F32, name="c2ps", tag="c2ps")
                         psf = ps.rearrange("p r w -> p (r w)")
                         plan = _c3plan(y0, nr, 6, ci_order=[2, 3, 5, 0, 1, 4])
                         for i, (o, u, s, ylo, yhi) in enumerate(plan):
                             out = psf if (ylo == y0 and yhi == y0 + nr) else \
                                 psf[:, (ylo - y0) * W:(yhi - y0) * W]
                             if u < 4:      # A-term: w_hi x (hi+lo) for ci=u
                                 lhsT = fwa_sb[:, u, o, :, cb_i * 128:(cb_i + 1) * 128]
                                 rhs = xi8[:, u, ylo + s:yhi + s,
                                           2 * (o % 3):2 * (o % 3) + 176].rearrange(
                                     "p h (w b) -> p b h w", b=2)
                             else:          # B-term: w_lo x hi for ci pair u-4
                                 t = u - 4
                                 lhsT = fwb_sb[:, t, o, :, cb_i * 128:(cb_i + 1) * 128]
                                 rhs = hi_pair(t, ylo + s, yhi + s, o % 3)
                             nc.tensor.matmul(out, lhsT, rhs, start=(i == 0),
                                              stop=False, perf_mode=DR)
                         nc.tensor.matmul(
                             psf, fwm_sb[:, cb_i * 128:(cb_i + 1) * 128],
                             im2v[:, y0:y0 + nr, 0:W], start=False, stop=True)
                         o_t = ev.tile([128, nn], F32, name="o_t", tag="o_t")
                         nc.scalar.activation(o_t, psf, AF.Relu,
                                              bias=cbc(CB_FT, cb_i),
                                              scale=cbc(CB_FS, cb_i))
                         nc.sync.dma_start(
                             out=out_d[cb_i * 128:(cb_i + 1) * 128,
                                       y0 * W:y0 * W + nn],
                             in_=o_t)
             pfw_es.close()
    nc.finalize()
    return nc


def _c3plan(y0, nr, n_u, ci_order=None):
    """Offset/unit plan for a 3x3 conv block: (offset, unit, row_shift, ylo, yhi)
    with dy==1 offsets first so the initial matmul covers the full region."""
    plan = []
    for dy, dx in [(1, 0), (1, 1), (1, 2), (0, 0), (0, 1), (0, 2),
                   (2, 0), (2, 1), (2, 2)]:
        s = dy - 1
        ylo = max(y0, -s); yhi = min(y0 + nr, H - s)
        if ylo >= yhi:
            continue
        for u in (ci_order or range(n_u)):
            plan.append((dy * 3 + dx, u, s, ylo, yhi))
    return plan


def kernel(**inputs):
    in_maps, b2 = _prep(inputs)
    key = ("nc", round(b2, 9))
    if key not in _CACHE:
        nc = bacc.Bacc("TRN2", target_bir_lowering=False, debug=False)
        _build(nc, b2)
        _CACHE[key] = nc
    nc = _CACHE[key]
    res = run_bass_kernel_spmd(nc, in_maps, list(range(B)))
    return np.stack([np.asarray(res.results[i]["out"], np.float32).reshape(C, H, W)
                     for i in range(B)])
